# revision 16
# baseline (speedup 1.0000x reference)
"""ADMM DC-layer kernel for Trainium2 (8 NeuronCores, data-parallel over batch).

Strategy
--------
Batch (1024) is sharded 128 samples per core; A and the 128x128 SMW-system
factorization are replicated (host-precomputed S^-1 and H = rho*S^-1*Gblock).

Math (per core, bs=128 samples):
  c0T   = (A @ r_n)^T stacked            one-time, contraction over N
  per ADMM step:
    sol  = S^-1 c0T + H zmuT             (two accumulating 128x128 matmuls)
    wT   = rho*zmuT - sol
    xT   = relu(rT_r + A_stack-chunks^T @ wT)     (64 chunk matmuls + epilogue)
    AxT  = AT1-chunks^T @ xT-chunks       (64 accumulating matmuls)
    z/u dual updates in normal [sample, 2M] layout (one small PE transpose)

Loads are host-packed, fully contiguous, and pipelined: G group slabs
(AT1 | rT_r | rT_i interleaved) stream in while the one-time c0 matmuls
consume them group by group; A_stack arrives last, just in time for step 1.
The xT store is split and overlapped with step-3 compute. Host pre-transposes
r_n and post-transposes x so every DMA is contiguous-innermost.

TRN2 gotchas honored here: at most 1 sync wait per instruction (use Bacc +
compile() for the legalization passes), no DVE op may read 2 PSUM operands,
DMA APs max 3 dims with contiguous innermost on both sides.
"""
import os
import sys

sys.path.insert(0, "/opt/trn_rl_repo")

import numpy as np

BATCH = 1024
M = 64
N = 8192
STEPS = 3
NCORES = 8
BS = BATCH // NCORES   # 128 samples per core
T = N // 128           # 64 chunks along N
G = 16                 # load groups
TG = T // G            # 4 chunks per group
GCOLS = 3 * TG * 128   # A_stack | rTr | rTi sub-blocks per group
EPS_DIV = 1e-12

_BUILD_CACHE = {}


def build_bass(rho, eps):
    """Build the per-core Bass program (identical on all cores)."""
    import concourse.bacc as bacc
    import concourse.tile as tile
    import concourse.mybir as mybir

    f32 = mybir.dt.float32
    Alu = mybir.AluOpType
    Act = mybir.ActivationFunctionType

    nc = bacc.Bacc("TRN2", target_bir_lowering=False)

    sm_d = nc.dram_tensor("sm", [128, 384], f32, kind="ExternalInput")  # HT|SinvT|I
    grp_d = [
        nc.dram_tensor(f"g{g}", [128, GCOLS], f32, kind="ExternalInput")
        for g in range(G)
    ]
    As_d = [
        nc.dram_tensor("As0", [128, N // 2], f32, kind="ExternalInput"),
        nc.dram_tensor("As1", [128, N // 2], f32, kind="ExternalInput"),
    ]
    y_d = nc.dram_tensor("y", [BS, 128], f32, kind="ExternalInput")
    u0_d = nc.dram_tensor("u0", [BS, 128], f32, kind="ExternalInput")

    xTo_d = nc.dram_tensor("xTo", [N, BS], f32, kind="ExternalOutput")
    uo_d = nc.dram_tensor("uo", [BS, 128], f32, kind="ExternalOutput")

    with tile.TileContext(nc) as tc:
        with (
            tc.tile_pool(name="big", bufs=1) as big,
            tc.tile_pool(name="small", bufs=1) as small,
            tc.tile_pool(name="ps", bufs=1, space="PSUM") as ps,
        ):
            # ---- loads, in pipeline order ----
            sm = small.tile([128, 384], f32, tag="sm")
            nc.sync.dma_start(sm[:], sm_d[:])
            y_sb = small.tile([BS, 128], f32, tag="y")
            nc.sync.dma_start(y_sb[:], y_d[:])
            u_sb = small.tile([BS, 128], f32, tag="u")
            nc.sync.dma_start(u_sb[:], u0_d[:])

            A_stack = big.tile([128, N], f32, tag="A_stack")
            grp = []
            for g in range(G):
                gt = big.tile([128, GCOLS], f32, tag=f"grp{g}")
                nc.sync.dma_start(gt[:], grp_d[g][:])
                grp.append(gt)
                if g == 7:
                    nc.sync.dma_start(A_stack[:, 0:N // 2], As_d[0][:])
                if g == 11:
                    nc.sync.dma_start(A_stack[:, N // 2:N], As_d[1][:])

            HT_sb = sm[:, 0:128]
            SiT_sb = sm[:, 128:256]
            I_sb = sm[:, 256:384]

            def at1(t):  # AT1 chunk t [128, 128] (from its group slab)
                g, i = divmod(t, TG)
                return grp[g][:, i * 128:(i + 1) * 128]

            def as_(t):  # A_stack chunk t
                return A_stack[:, t * 128:(t + 1) * 128]

            def rtr(t, w=1):  # rTr chunks t..t+w-1 (must stay in one group)
                g, i = divmod(t, TG)
                return grp[g][:, TG * 128 + i * 128: TG * 128 + (i + w) * 128]

            def rti(t):
                g, i = divmod(t, TG)
                return grp[g][:, 2 * TG * 128 + i * 128: 2 * TG * 128 + (i + 1) * 128]

            xT = big.tile([128, N], f32, tag="xT")

            # hoisted step-1 preamble: zmuT and the H-part of sol only need
            # y, u, HT — they run during the load/c0 phase
            zmu = small.tile([BS, 128], f32, tag="zmu")
            nc.vector.tensor_sub(zmu[:], y_sb[:], u_sb[:])  # z0 - u0
            tp1 = ps.tile([128, 128], f32, tag="tp", bufs=2)
            nc.tensor.transpose(tp1[:], zmu[:], I_sb)
            zmuT = small.tile([128, 128], f32, tag="zmuT", bufs=2)
            nc.vector.tensor_copy(zmuT[:], tp1[:])
            sol = ps.tile([128, BS], f32, tag="sol", bufs=1)
            nc.tensor.matmul(sol[:], HT_sb, zmuT[:], start=True, stop=False)

            # ---- c0T accumulation, pipelined per 768KB group slab ----
            P1 = ps.tile([128, BS], f32, tag="pc", bufs=3)
            P2 = ps.tile([128, BS], f32, tag="pc", bufs=3)
            for g in range(G):
                for i in range(TG):
                    t = g * TG + i
                    nc.tensor.matmul(P2[:], at1(t), rti(t),
                                     start=(t == 0), stop=(t == T - 1))
                for i in range(TG):
                    t = g * TG + i
                    nc.tensor.matmul(P1[:], at1(t), rtr(t),
                                     start=(t == 0), stop=(t == T - 1))
            c0T = small.tile([128, BS], f32, tag="c0T")
            p2s = small.tile([128, BS], f32, tag="p2s")
            nc.scalar.copy(p2s[:], P2[:])
            # c0_r^T = P1[:64] - P2[64:], c0_i^T = P2[:64] + P1[64:]
            nc.vector.tensor_sub(c0T[0:64, :], P1[0:64, :], p2s[64:128, :])
            nc.vector.tensor_add(c0T[64:128, :], P1[64:128, :], p2s[0:64, :])

            # ---- ADMM steps ----
            for step in range(STEPS):
                last = step == STEPS - 1
                # finish sol = H @ zmuT + S^-1 @ c0T (H-part already queued)
                nc.tensor.matmul(sol[:], SiT_sb, c0T[:], start=False, stop=True)

                # wT = rho * zmuT - sol
                wT = small.tile([128, 128], f32, tag="wT", bufs=2)
                nc.vector.scalar_tensor_tensor(
                    wT[:], zmuT[:], float(rho), sol[:],
                    op0=Alu.mult, op1=Alu.subtract,
                )

                # x-update + Ax accumulation (two accumulators decouple the
                # relu->ax dependency chain), 4 chunks at a time
                ax_e = ps.tile([128, BS], f32, tag="ax", bufs=2)
                ax_o = ps.tile([128, BS], f32, tag="ax", bufs=2)
                NQ = T // 4
                for q in range(NQ):
                    pc = ps.tile([128, 512], f32, tag="pc", bufs=3)
                    for i in range(4):
                        t = 4 * q + i
                        nc.tensor.matmul(
                            pc[:, i * 128:(i + 1) * 128],
                            as_(t), wT[:],
                            start=True, stop=True,
                        )
                    sl = slice(q * 512, (q + 1) * 512)
                    nc.vector.tensor_add(xT[:, sl], pc[:], rtr(4 * q, 4))
                    if q % 2 == 0:
                        nc.scalar.activation(xT[:, sl], xT[:, sl], Act.Relu)
                    else:
                        nc.vector.tensor_scalar_max(xT[:, sl], xT[:, sl], 0.0)
                    axp = ax_e if q % 2 == 0 else ax_o
                    for i in range(4):
                        t = 4 * q + i
                        nc.tensor.matmul(
                            axp[:], at1(t), xT[:, t * 128:(t + 1) * 128],
                            start=(q < 2 and i == 0),
                            stop=(q >= NQ - 2 and i == 3),
                        )
                    if last and q % 2 == 1:
                        # overlap the output store with step-3 compute
                        lo = (q - 1) * 512
                        nc.sync.dma_start(
                            xTo_d[lo:lo + 1024, :].rearrange("(t p) s -> p t s", p=128),
                            xT[:, lo:lo + 1024].rearrange("p (t s) -> p t s", s=BS),
                        )

                # Ax = ax_e + ax_o via two accumulating PE transposes
                axe_sb = small.tile([128, BS], f32, tag="axe", bufs=2)
                nc.scalar.copy(axe_sb[:], ax_e[:])
                axo_sb = small.tile([128, BS], f32, tag="axo", bufs=2)
                nc.vector.tensor_copy(axo_sb[:], ax_o[:])
                ax_n = ps.tile([128, 128], f32, tag="tp", bufs=2)
                nc.tensor.matmul(ax_n[:], axe_sb[:], I_sb, is_transpose=True,
                                 start=True, stop=False)
                nc.tensor.matmul(ax_n[:], axo_sb[:], I_sb, is_transpose=True,
                                 start=False, stop=True)

                # dual updates (normal layout)
                uy = small.tile([BS, 128], f32, tag="uy", bufs=2)
                nc.vector.tensor_sub(uy[:], u_sb[:], y_sb[:])
                v = small.tile([BS, 128], f32, tag="v", bufs=2)
                nc.vector.tensor_add(v[:], ax_n[:], uy[:])
                vsq = small.tile([BS, 128], f32, tag="vsq", bufs=2)
                nrm2 = small.tile([BS, 1], f32, tag="nrm2", bufs=2)
                nc.vector.scalar_tensor_tensor(
                    vsq[:], v[:], 1.0, v[:],
                    op0=Alu.mult, op1=Alu.mult, accum_out=nrm2[:],
                )
                nrm = small.tile([BS, 1], f32, tag="nrm", bufs=2)
                nc.scalar.sqrt(nrm[:], nrm2[:])
                nc.vector.tensor_scalar_add(nrm[:], nrm[:], EPS_DIV)
                rec = small.tile([BS, 1], f32, tag="rec", bufs=2)
                nc.vector.reciprocal(rec[:], nrm[:])
                fs = small.tile([BS, 1], f32, tag="fs", bufs=2)
                nc.vector.tensor_scalar(
                    fs[:], rec[:], float(eps), 1.0, op0=Alu.mult, op1=Alu.min,
                )
                # z = y + v * f
                z_new = small.tile([BS, 128], f32, tag="z", bufs=2)
                nc.vector.scalar_tensor_tensor(
                    z_new[:], v[:], fs[:], y_sb[:], op0=Alu.mult, op1=Alu.add,
                )
                # u += Ax - z
                t1 = small.tile([BS, 128], f32, tag="t1", bufs=2)
                nc.vector.tensor_sub(t1[:], ax_n[:], z_new[:])
                nc.vector.tensor_add(u_sb[:], u_sb[:], t1[:])
                if not last:
                    # rolling preamble for the next step
                    nc.vector.tensor_sub(zmu[:], z_new[:], u_sb[:])
                    tp1 = ps.tile([128, 128], f32, tag="tp", bufs=2)
                    nc.tensor.transpose(tp1[:], zmu[:], I_sb)
                    zmuT = small.tile([128, 128], f32, tag="zmuT", bufs=2)
                    nc.vector.tensor_copy(zmuT[:], tp1[:])
                    sol = ps.tile([128, BS], f32, tag="sol", bufs=1)
                    nc.tensor.matmul(sol[:], HT_sb, zmuT[:], start=True, stop=False)

            nc.sync.dma_start(uo_d[:], u_sb[:])

    nc.compile()
    return nc


def _host_prep(A, log_rho, log_epsilon):
    rho = float(np.exp(np.float64(np.asarray(log_rho))))
    eps = float(np.exp(np.float64(np.asarray(log_epsilon))))
    Ar = np.asarray(A[0], np.float64)
    Ai = np.asarray(A[1], np.float64)
    Gr = Ar @ Ar.T + Ai @ Ai.T
    Gi = Ai @ Ar.T - Ar @ Ai.T
    Sr = np.eye(M) / (rho + EPS_DIV) + Gr
    Sb = np.block([[Sr, -Gi], [Gi, Sr]])
    Sinv = np.linalg.inv(Sb)
    Gb = np.block([[Gr, -Gi], [Gi, Gr]])
    H = rho * (Sinv @ Gb)
    HT = np.ascontiguousarray(H.T, dtype=np.float32)
    SinvT = np.ascontiguousarray(Sinv.T, dtype=np.float32)
    return rho, eps, HT, SinvT


def _chunked_T(rT):
    """[N, BS] -> [128, N] with col t*BS+s = rT[t*128+p, s]."""
    return np.ascontiguousarray(
        rT.reshape(T, 128, BS).transpose(1, 0, 2).reshape(128, T * BS)
    )


def make_in_maps(r_n, y, u_in, A, log_rho, log_epsilon):
    rho, eps, HT, SinvT = _host_prep(A, log_rho, log_epsilon)
    A_f = np.asarray(A, np.float32)
    A_stack = np.ascontiguousarray(A_f.reshape(128, N))
    AT1_ch = _chunked_T(np.concatenate([A_f[0].T, A_f[1].T], axis=1))
    I128 = np.eye(128, dtype=np.float32)
    sm = np.ascontiguousarray(
        np.concatenate([HT, SinvT, I128], axis=1), np.float32
    )
    r_n = np.asarray(r_n, np.float32)
    y = np.asarray(y, np.float32)
    u_in = np.asarray(u_in, np.float32)

    in_maps = []
    for c in range(NCORES):
        sl = slice(c * BS, (c + 1) * BS)
        rTr_ch = _chunked_T(np.ascontiguousarray(r_n[sl, 0, :].T))
        rTi_ch = _chunked_T(np.ascontiguousarray(r_n[sl, 1, :].T))
        im = {
            "sm": sm,
            "As0": np.ascontiguousarray(A_stack[:, :N // 2]),
            "As1": np.ascontiguousarray(A_stack[:, N // 2:]),
            "y": np.ascontiguousarray(y[sl].reshape(BS, 128)),
            "u0": np.ascontiguousarray(u_in[sl].reshape(BS, 128)),
        }
        W = TG * 128
        for g in range(G):
            gs = slice(g * W, (g + 1) * W)
            im[f"g{g}"] = np.ascontiguousarray(np.concatenate(
                [AT1_ch[:, gs], rTr_ch[:, gs], rTi_ch[:, gs]], axis=1
            ))
        in_maps.append(im)
    return rho, eps, in_maps


def assemble_outputs(out_maps):
    xs = []
    us = []
    for c in range(NCORES):
        xs.append(np.asarray(out_maps[c]["xTo"]).T)  # [BS, N]
        us.append(np.asarray(out_maps[c]["uo"]).reshape(BS, 2, M))
    x_r = np.concatenate(xs, axis=0)  # [1024, N]
    x = np.stack([x_r, np.zeros_like(x_r)], axis=1)
    u = np.concatenate(us, axis=0)
    return x.astype(np.float32), u.astype(np.float32)


def kernel(r_n, y, u_in, A, log_rho, log_epsilon, _trace=False):
    from concourse.bass_utils import run_bass_kernel_spmd

    rho, eps, in_maps = make_in_maps(r_n, y, u_in, A, log_rho, log_epsilon)
    key = (round(rho, 12), round(eps, 12))
    if key not in _BUILD_CACHE:
        _BUILD_CACHE[key] = build_bass(rho, eps)
    nc = _BUILD_CACHE[key]
    res = run_bass_kernel_spmd(
        nc, in_maps, core_ids=list(range(NCORES)), trace=_trace,
    )
    x, u = assemble_outputs(res.results)
    if _trace:
        kernel._last_exec_time_ns = res.exec_time_ns
        kernel._last_results = res
    return x, u


# revision 17
# speedup vs baseline: 1.0158x; 1.0158x over previous
"""ADMM DC-layer kernel for Trainium2 (8 NeuronCores, data-parallel over batch).

Strategy
--------
Batch (1024) is sharded 128 samples per core; A and the 128x128 SMW-system
factorization are replicated (host-precomputed S^-1 and H = rho*S^-1*Gblock).

Math (per core, bs=128 samples):
  c0T   = (A @ r_n)^T stacked            one-time, contraction over N
  per ADMM step:
    sol  = S^-1 c0T + H zmuT             (two accumulating 128x128 matmuls)
    wT   = rho*zmuT - sol
    xT   = relu(rT_r + A_stack-chunks^T @ wT)     (64 chunk matmuls + epilogue)
    AxT  = AT1-chunks^T @ xT-chunks       (64 accumulating matmuls)
    z/u dual updates in normal [sample, 2M] layout (one small PE transpose)

Loads are host-packed, fully contiguous, and pipelined: G group slabs
(AT1 | rT_r | rT_i interleaved) stream in while the one-time c0 matmuls
consume them group by group; A_stack arrives last, just in time for step 1.
The xT store is split and overlapped with step-3 compute. Host pre-transposes
r_n and post-transposes x so every DMA is contiguous-innermost.

TRN2 gotchas honored here: at most 1 sync wait per instruction (use Bacc +
compile() for the legalization passes), no DVE op may read 2 PSUM operands,
DMA APs max 3 dims with contiguous innermost on both sides.
"""
import os
import sys

sys.path.insert(0, "/opt/trn_rl_repo")

import numpy as np

BATCH = 1024
M = 64
N = 8192
STEPS = 3
NCORES = 8
BS = BATCH // NCORES   # 128 samples per core
T = N // 128           # 64 chunks along N
G = 16                 # load groups
TG = T // G            # 4 chunks per group
GCOLS = 3 * TG * 128   # A_stack | rTr | rTi sub-blocks per group
EPS_DIV = 1e-12

_BUILD_CACHE = {}


def build_bass(rho, eps):
    """Build the per-core Bass program (identical on all cores)."""
    import concourse.bacc as bacc
    import concourse.tile as tile
    import concourse.mybir as mybir

    f32 = mybir.dt.float32
    Alu = mybir.AluOpType
    Act = mybir.ActivationFunctionType

    nc = bacc.Bacc("TRN2", target_bir_lowering=False)

    sm_d = nc.dram_tensor("sm", [128, 512], f32, kind="ExternalInput")  # HT|SinvT|I|ONES
    grp_d = [
        nc.dram_tensor(f"g{g}", [128, GCOLS], f32, kind="ExternalInput")
        for g in range(G)
    ]
    As_d = [
        nc.dram_tensor(f"As{k}", [128, N // 4], f32, kind="ExternalInput")
        for k in range(4)
    ]
    yT_d = nc.dram_tensor("yT", [128, BS], f32, kind="ExternalInput")
    uT_d = nc.dram_tensor("u0T", [128, BS], f32, kind="ExternalInput")

    xTo_d = nc.dram_tensor("xTo", [N, BS], f32, kind="ExternalOutput")
    uo_d = nc.dram_tensor("uoT", [128, BS], f32, kind="ExternalOutput")

    with tile.TileContext(nc) as tc:
        with (
            tc.tile_pool(name="big", bufs=1) as big,
            tc.tile_pool(name="small", bufs=1) as small,
            tc.tile_pool(name="ps", bufs=1, space="PSUM") as ps,
        ):
            # ---- loads, in pipeline order ----
            sm = small.tile([128, 512], f32, tag="sm")
            nc.sync.dma_start(sm[:], sm_d[:])
            yT = small.tile([128, BS], f32, tag="yT")
            nc.sync.dma_start(yT[:], yT_d[:])
            u0T = small.tile([128, BS], f32, tag="u0T")
            nc.sync.dma_start(u0T[:], uT_d[:])

            grp = []
            for g in range(G):
                gt = big.tile([128, GCOLS], f32, tag=f"grp{g}")
                nc.sync.dma_start(gt[:], grp_d[g][:])
                grp.append(gt)
            A_stack = big.tile([128, N], f32, tag="A_stack")
            for k in range(4):
                nc.sync.dma_start(
                    A_stack[:, k * (N // 4):(k + 1) * (N // 4)], As_d[k][:])

            HT_sb = sm[:, 0:128]
            SiT_sb = sm[:, 128:256]
            ones_col = sm[:, 384:385]        # [128, 1] of ones
            ones_row = sm[0:1, 384:512]      # [1, 128] of ones

            def at1(t):  # AT1 chunk t [128, 128] (from its group slab)
                g, i = divmod(t, TG)
                return grp[g][:, i * 128:(i + 1) * 128]

            def as_(t):  # A_stack chunk t
                return A_stack[:, t * 128:(t + 1) * 128]

            def rtr(t, w=1):  # rTr chunks t..t+w-1 (must stay in one group)
                g, i = divmod(t, TG)
                return grp[g][:, TG * 128 + i * 128: TG * 128 + (i + w) * 128]

            def rti(t):
                g, i = divmod(t, TG)
                return grp[g][:, 2 * TG * 128 + i * 128: 2 * TG * 128 + (i + 1) * 128]

            xT = big.tile([128, N], f32, tag="xT")

            # hoisted step-1 preamble (all in transposed space, no transposes)
            uyT = small.tile([128, BS], f32, tag="uyT", bufs=2)
            nc.vector.tensor_sub(uyT[:], u0T[:], yT[:])
            zmuT = small.tile([128, BS], f32, tag="zmuT", bufs=2)
            nc.vector.tensor_scalar_mul(zmuT[:], uyT[:], -1.0)  # y - u0
            sol = ps.tile([128, BS], f32, tag="sol", bufs=1)
            nc.tensor.matmul(sol[:], HT_sb, zmuT[:], start=True, stop=False)

            # ---- c0T accumulation, pipelined per 768KB group slab ----
            P1 = ps.tile([128, BS], f32, tag="pc", bufs=3)
            P2 = ps.tile([128, BS], f32, tag="pc", bufs=3)
            for g in range(G):
                for i in range(TG):
                    t = g * TG + i
                    nc.tensor.matmul(P2[:], at1(t), rti(t),
                                     start=(t == 0), stop=(t == T - 1))
                for i in range(TG):
                    t = g * TG + i
                    nc.tensor.matmul(P1[:], at1(t), rtr(t),
                                     start=(t == 0), stop=(t == T - 1))
            c0T = small.tile([128, BS], f32, tag="c0T")
            p2s = small.tile([128, BS], f32, tag="p2s")
            nc.scalar.copy(p2s[:], P2[:])
            # c0_r^T = P1[:64] - P2[64:], c0_i^T = P2[:64] + P1[64:]
            nc.vector.tensor_sub(c0T[0:64, :], P1[0:64, :], p2s[64:128, :])
            nc.vector.tensor_add(c0T[64:128, :], P1[64:128, :], p2s[0:64, :])

            # ---- ADMM steps ----
            for step in range(STEPS):
                last = step == STEPS - 1
                # finish sol = H @ zmuT + S^-1 @ c0T (H-part already queued)
                nc.tensor.matmul(sol[:], SiT_sb, c0T[:], start=False, stop=True)

                # wT = rho * zmuT - sol
                wT = small.tile([128, BS], f32, tag="wT", bufs=2)
                nc.vector.scalar_tensor_tensor(
                    wT[:], zmuT[:], float(rho), sol[:],
                    op0=Alu.mult, op1=Alu.subtract,
                )

                # x-update + Ax accumulation, 4 chunks at a time
                ax_ps = ps.tile([128, BS], f32, tag="ax", bufs=2)
                NQ = T // 4
                for q in range(NQ):
                    pc = ps.tile([128, 512], f32, tag="pc", bufs=3)
                    for i in range(4):
                        t = 4 * q + i
                        nc.tensor.matmul(
                            pc[:, i * 128:(i + 1) * 128],
                            as_(t), wT[:],
                            start=True, stop=True,
                        )
                    sl = slice(q * 512, (q + 1) * 512)
                    nc.vector.tensor_add(xT[:, sl], pc[:], rtr(4 * q, 4))
                    if q % 2 == 0:
                        nc.scalar.activation(xT[:, sl], xT[:, sl], Act.Relu)
                    else:
                        nc.vector.tensor_scalar_max(xT[:, sl], xT[:, sl], 0.0)
                    for i in range(4):
                        t = 4 * q + i
                        nc.tensor.matmul(
                            ax_ps[:], at1(t), xT[:, t * 128:(t + 1) * 128],
                            start=(t == 0), stop=(t == T - 1),
                        )
                    if last and q % 2 == 1:
                        # overlap the output store with step-3 compute
                        lo = (q - 1) * 512
                        nc.sync.dma_start(
                            xTo_d[lo:lo + 1024, :].rearrange("(t p) s -> p t s", p=128),
                            xT[:, lo:lo + 1024].rearrange("p (t s) -> p t s", s=BS),
                        )

                # dual update, entirely in transposed space:
                #   v = Ax + u - y;  f = min(1, eps/(||v||+tiny))
                #   u' = v*(1-f);  zmu' = y - u' + v*f = y + v*(2f-1)
                vT = small.tile([128, BS], f32, tag="vT", bufs=2)
                nc.vector.tensor_add(vT[:], ax_ps[:], uyT[:])
                sq = small.tile([128, BS], f32, tag="sq", bufs=2)
                nc.vector.tensor_mul(sq[:], vT[:], vT[:])
                nrm2 = ps.tile([1, BS], f32, tag="tp", bufs=2)
                nc.tensor.matmul(nrm2[:], ones_col, sq[:], start=True, stop=True)
                nrm = small.tile([1, BS], f32, tag="nrm", bufs=2)
                nc.scalar.sqrt(nrm[:], nrm2[:])
                rec = small.tile([1, BS], f32, tag="rec", bufs=2)
                nc.vector.tensor_scalar_add(nrm[:], nrm[:], EPS_DIV)
                nc.vector.reciprocal(rec[:], nrm[:])
                hg = small.tile([1, 2 * BS], f32, tag="hg", bufs=2)
                # f = min(eps*rec, 1);  h = 2f-1 ; g = 1-f
                fs = small.tile([1, BS], f32, tag="fs", bufs=2)
                nc.vector.tensor_scalar(
                    fs[:], rec[:], float(eps), 1.0, op0=Alu.mult, op1=Alu.min,
                )
                nc.vector.tensor_scalar(
                    hg[:, 0:BS], fs[:], 2.0, 1.0, op0=Alu.mult, op1=Alu.subtract,
                )
                nc.vector.tensor_scalar(
                    hg[:, BS:2 * BS], fs[:], -1.0, 1.0, op0=Alu.mult, op1=Alu.add,
                )
                bc = ps.tile([128, 2 * BS], f32, tag="tp", bufs=2)
                nc.tensor.matmul(bc[:], ones_row, hg[:], start=True, stop=True)
                # u' = v*(1-f)
                uT_new = small.tile([128, BS], f32, tag="uT", bufs=2)
                nc.vector.tensor_mul(uT_new[:], vT[:], bc[:, BS:2 * BS])
                if last:
                    nc.sync.dma_start(uo_d[:], uT_new[:])
                else:
                    uyT = small.tile([128, BS], f32, tag="uyT", bufs=2)
                    nc.vector.tensor_sub(uyT[:], uT_new[:], yT[:])
                    # zmu' = y + v*(2f-1)
                    zmT = small.tile([128, BS], f32, tag="zmT", bufs=2)
                    nc.vector.tensor_mul(zmT[:], vT[:], bc[:, 0:BS])
                    zmuT = small.tile([128, BS], f32, tag="zmuT", bufs=2)
                    nc.vector.tensor_add(zmuT[:], zmT[:], yT[:])
                    sol = ps.tile([128, BS], f32, tag="sol", bufs=1)
                    nc.tensor.matmul(sol[:], HT_sb, zmuT[:], start=True, stop=False)

    nc.compile()
    return nc


def _host_prep(A, log_rho, log_epsilon):
    rho = float(np.exp(np.float64(np.asarray(log_rho))))
    eps = float(np.exp(np.float64(np.asarray(log_epsilon))))
    Ar = np.asarray(A[0], np.float64)
    Ai = np.asarray(A[1], np.float64)
    Gr = Ar @ Ar.T + Ai @ Ai.T
    Gi = Ai @ Ar.T - Ar @ Ai.T
    Sr = np.eye(M) / (rho + EPS_DIV) + Gr
    Sb = np.block([[Sr, -Gi], [Gi, Sr]])
    Sinv = np.linalg.inv(Sb)
    Gb = np.block([[Gr, -Gi], [Gi, Gr]])
    H = rho * (Sinv @ Gb)
    HT = np.ascontiguousarray(H.T, dtype=np.float32)
    SinvT = np.ascontiguousarray(Sinv.T, dtype=np.float32)
    return rho, eps, HT, SinvT


def _chunked_T(rT):
    """[N, BS] -> [128, N] with col t*BS+s = rT[t*128+p, s]."""
    return np.ascontiguousarray(
        rT.reshape(T, 128, BS).transpose(1, 0, 2).reshape(128, T * BS)
    )


def make_in_maps(r_n, y, u_in, A, log_rho, log_epsilon):
    rho, eps, HT, SinvT = _host_prep(A, log_rho, log_epsilon)
    A_f = np.asarray(A, np.float32)
    A_stack = np.ascontiguousarray(A_f.reshape(128, N))
    AT1_ch = _chunked_T(np.concatenate([A_f[0].T, A_f[1].T], axis=1))
    I128 = np.eye(128, dtype=np.float32)
    sm = np.ascontiguousarray(np.concatenate(
        [HT, SinvT, I128, np.ones((128, 128), np.float32)], axis=1
    ), np.float32)
    r_n = np.asarray(r_n, np.float32)
    y = np.asarray(y, np.float32)
    u_in = np.asarray(u_in, np.float32)

    in_maps = []
    for c in range(NCORES):
        sl = slice(c * BS, (c + 1) * BS)
        rTr_ch = _chunked_T(np.ascontiguousarray(r_n[sl, 0, :].T))
        rTi_ch = _chunked_T(np.ascontiguousarray(r_n[sl, 1, :].T))
        im = {
            "sm": sm,
            "yT": np.ascontiguousarray(y[sl].reshape(BS, 128).T),
            "u0T": np.ascontiguousarray(u_in[sl].reshape(BS, 128).T),
        }
        for k in range(4):
            im[f"As{k}"] = np.ascontiguousarray(
                A_stack[:, k * (N // 4):(k + 1) * (N // 4)])
        W = TG * 128
        for g in range(G):
            gs = slice(g * W, (g + 1) * W)
            im[f"g{g}"] = np.ascontiguousarray(np.concatenate(
                [AT1_ch[:, gs], rTr_ch[:, gs], rTi_ch[:, gs]], axis=1
            ))
        in_maps.append(im)
    return rho, eps, in_maps


def assemble_outputs(out_maps):
    xs = []
    us = []
    for c in range(NCORES):
        xs.append(np.asarray(out_maps[c]["xTo"]).T)  # [BS, N]
        us.append(np.asarray(out_maps[c]["uoT"]).T.reshape(BS, 2, M))
    x_r = np.concatenate(xs, axis=0)  # [1024, N]
    x = np.stack([x_r, np.zeros_like(x_r)], axis=1)
    u = np.concatenate(us, axis=0)
    return x.astype(np.float32), u.astype(np.float32)


def kernel(r_n, y, u_in, A, log_rho, log_epsilon, _trace=False):
    from concourse.bass_utils import run_bass_kernel_spmd

    rho, eps, in_maps = make_in_maps(r_n, y, u_in, A, log_rho, log_epsilon)
    key = (round(rho, 12), round(eps, 12))
    if key not in _BUILD_CACHE:
        _BUILD_CACHE[key] = build_bass(rho, eps)
    nc = _BUILD_CACHE[key]
    res = run_bass_kernel_spmd(
        nc, in_maps, core_ids=list(range(NCORES)), trace=_trace,
    )
    x, u = assemble_outputs(res.results)
    if _trace:
        kernel._last_exec_time_ns = res.exec_time_ns
        kernel._last_results = res
    return x, u


# revision 19
# speedup vs baseline: 1.0500x; 1.0337x over previous
"""ADMM DC-layer kernel for Trainium2 (8 NeuronCores, data-parallel over batch).

Strategy
--------
Batch (1024) is sharded 128 samples per core; A and the 128x128 SMW-system
factorization are replicated (host-precomputed S^-1 and H = rho*S^-1*Gblock).

Math (per core, bs=128 samples), everything in "transposed" layout
(N / 2M on partitions, samples on the free axis; no on-device transposes):
  c0T   = (A @ r_n)^T stacked            one-time, contraction over N
  per ADMM step:
    sol  = H zmuT + S^-1 c0T             (two accumulating 128x128 matmuls)
    wT   = rho*zmuT - sol
    xT   = relu(rT_r + A_stack-chunks^T @ wT)     (64 chunk matmuls + epilogue)
    AxT  = AT1-chunks^T @ xT-chunks       (64 accumulating matmuls)
    dual update via u' = v(1-f), zmu' = y + v(2f-1), v = AxT + (u-y):
    per-sample ||v|| by ones-vector matmul, f broadcast by a K=1 matmul

Loads are host-packed, fully contiguous, and pipelined: 768KB group slabs
(AT1 | rT_r | rT_i) stream in while the one-time c0 matmuls consume them
group by group; A_stack arrives last, just in time for step 1's x-update.
The xT store is split and overlapped with step-3 compute. Host pre-transposes
r_n / y / u and post-transposes x / u so every DMA is contiguous-innermost.

TRN2 gotchas honored here: at most 1 sync wait per instruction (use Bacc +
compile() for the legalization passes), no DVE op may read 2 PSUM operands,
DMA APs max 3 dims with contiguous innermost on both sides.
"""
import os
import sys

sys.path.insert(0, "/opt/trn_rl_repo")

import numpy as np

BATCH = 1024
M = 64
N = 8192
STEPS = 3
NCORES = 8
BS = BATCH // NCORES   # 128 samples per core
T = N // 128           # 64 chunks along N
G = 16                 # load groups
TG = T // G            # 4 chunks per group
GCOLS = 3 * TG * 128   # A_stack | rTr | rTi sub-blocks per group
EPS_DIV = 1e-12

_BUILD_CACHE = {}


def build_bass(rho, eps):
    """Build the per-core Bass program (identical on all cores)."""
    import concourse.bacc as bacc
    import concourse.tile as tile
    import concourse.mybir as mybir

    f32 = mybir.dt.float32
    Alu = mybir.AluOpType
    Act = mybir.ActivationFunctionType

    nc = bacc.Bacc("TRN2", target_bir_lowering=False)

    sm_d = nc.dram_tensor("sm", [128, 512], f32, kind="ExternalInput")  # HT|SinvT|I|ONES
    grp_d = [
        nc.dram_tensor(f"g{g}", [128, GCOLS], f32, kind="ExternalInput")
        for g in range(G)
    ]
    As_d = [
        nc.dram_tensor(f"As{k}", [128, N // 4], f32, kind="ExternalInput")
        for k in range(4)
    ]
    yT_d = nc.dram_tensor("yT", [128, BS], f32, kind="ExternalInput")
    uT_d = nc.dram_tensor("u0T", [128, BS], f32, kind="ExternalInput")

    xTo_d = nc.dram_tensor("xTo", [N, BS], f32, kind="ExternalOutput")
    uo_d = nc.dram_tensor("uoT", [128, BS], f32, kind="ExternalOutput")

    with tile.TileContext(nc) as tc:
        with (
            tc.tile_pool(name="big", bufs=1) as big,
            tc.tile_pool(name="small", bufs=1) as small,
            tc.tile_pool(name="ps", bufs=1, space="PSUM") as ps,
        ):
            # ---- loads, in pipeline order ----
            sm = small.tile([128, 512], f32, tag="sm")
            nc.sync.dma_start(sm[:], sm_d[:])
            yT = small.tile([128, BS], f32, tag="yT")
            nc.sync.dma_start(yT[:], yT_d[:])
            u0T = small.tile([128, BS], f32, tag="u0T")
            nc.sync.dma_start(u0T[:], uT_d[:])

            grp = []
            for g in range(G):
                gt = big.tile([128, GCOLS], f32, tag=f"grp{g}")
                nc.sync.dma_start(gt[:], grp_d[g][:])
                grp.append(gt)
            A_stack = big.tile([128, N], f32, tag="A_stack")
            for k in range(4):
                nc.sync.dma_start(
                    A_stack[:, k * (N // 4):(k + 1) * (N // 4)], As_d[k][:])

            HT_sb = sm[:, 0:128]
            SiT_sb = sm[:, 128:256]
            ones_col = sm[:, 384:385]        # [128, 1] of ones
            ones_row = sm[0:1, 384:512]      # [1, 128] of ones

            def at1(t):  # AT1 chunk t [128, 128] (from its group slab)
                g, i = divmod(t, TG)
                return grp[g][:, i * 128:(i + 1) * 128]

            def as_(t):  # A_stack chunk t
                return A_stack[:, t * 128:(t + 1) * 128]

            def rtr(t, w=1):  # rTr chunks t..t+w-1 (must stay in one group)
                g, i = divmod(t, TG)
                return grp[g][:, TG * 128 + i * 128: TG * 128 + (i + w) * 128]

            def rti(t):
                g, i = divmod(t, TG)
                return grp[g][:, 2 * TG * 128 + i * 128: 2 * TG * 128 + (i + 1) * 128]

            xT = big.tile([128, N], f32, tag="xT")

            # hoisted step-1 preamble (all in transposed space, no transposes)
            uyT = small.tile([128, BS], f32, tag="uyT", bufs=2)
            nc.vector.tensor_sub(uyT[:], u0T[:], yT[:])
            zmuT = small.tile([128, BS], f32, tag="zmuT", bufs=2)
            nc.vector.tensor_scalar_mul(zmuT[:], uyT[:], -1.0)  # y - u0
            sol = ps.tile([128, BS], f32, tag="sol", bufs=1)
            nc.tensor.matmul(sol[:], HT_sb, zmuT[:], start=True, stop=False)

            # ---- c0T accumulation, pipelined per 768KB group slab ----
            P1 = ps.tile([128, BS], f32, tag="pc", bufs=3)
            P2 = ps.tile([128, BS], f32, tag="pc", bufs=3)
            for g in range(G):
                for i in range(TG):
                    t = g * TG + i
                    nc.tensor.matmul(P2[:], at1(t), rti(t),
                                     start=(t == 0), stop=(t == T - 1))
                for i in range(TG):
                    t = g * TG + i
                    nc.tensor.matmul(P1[:], at1(t), rtr(t),
                                     start=(t == 0), stop=(t == T - 1))
            c0T = small.tile([128, BS], f32, tag="c0T")
            p2s = small.tile([128, BS], f32, tag="p2s")
            nc.scalar.copy(p2s[:], P2[:])
            # c0_r^T = P1[:64] - P2[64:], c0_i^T = P2[:64] + P1[64:]
            nc.vector.tensor_sub(c0T[0:64, :], P1[0:64, :], p2s[64:128, :])
            nc.vector.tensor_add(c0T[64:128, :], P1[64:128, :], p2s[0:64, :])

            # ---- ADMM steps ----
            for step in range(STEPS):
                last = step == STEPS - 1
                # finish sol = H @ zmuT + S^-1 @ c0T (H-part already queued)
                nc.tensor.matmul(sol[:], SiT_sb, c0T[:], start=False, stop=True)

                # wT = rho * zmuT - sol
                wT = small.tile([128, BS], f32, tag="wT", bufs=2)
                nc.vector.scalar_tensor_tensor(
                    wT[:], zmuT[:], float(rho), sol[:],
                    op0=Alu.mult, op1=Alu.subtract,
                )

                # x-update + Ax accumulation, 4 chunks at a time
                ax_ps = ps.tile([128, BS], f32, tag="ax", bufs=2)
                NQ = T // 4
                for q in range(NQ):
                    pc = ps.tile([128, 512], f32, tag="pc", bufs=3)
                    for i in range(4):
                        t = 4 * q + i
                        nc.tensor.matmul(
                            pc[:, i * 128:(i + 1) * 128],
                            as_(t), wT[:],
                            start=True, stop=True,
                        )
                    if q < NQ - 1:
                        sl = slice(q * 512, (q + 1) * 512)
                        nc.vector.tensor_add(xT[:, sl], pc[:], rtr(4 * q, 4))
                        if q % 2 == 0:
                            nc.scalar.activation(xT[:, sl], xT[:, sl], Act.Relu)
                        else:
                            nc.vector.tensor_scalar_max(xT[:, sl], xT[:, sl], 0.0)
                        for i in range(4):
                            t = 4 * q + i
                            nc.tensor.matmul(
                                ax_ps[:], at1(t), xT[:, t * 128:(t + 1) * 128],
                                start=(t == 0), stop=False,
                            )
                    else:
                        # last group: per-chunk epilogue shortens the tail
                        # that gates the dual update
                        for i in range(4):
                            t = 4 * q + i
                            tsl = slice(t * 128, (t + 1) * 128)
                            nc.vector.tensor_add(
                                xT[:, tsl], pc[:, i * 128:(i + 1) * 128], rtr(t))
                            if i % 2 == 0:
                                nc.scalar.activation(xT[:, tsl], xT[:, tsl], Act.Relu)
                            else:
                                nc.vector.tensor_scalar_max(xT[:, tsl], xT[:, tsl], 0.0)
                            nc.tensor.matmul(
                                ax_ps[:], at1(t), xT[:, tsl],
                                start=False, stop=(t == T - 1),
                            )
                    if last and q % 2 == 1:
                        # overlap the output store with step-3 compute
                        lo = (q - 1) * 512
                        nc.sync.dma_start(
                            xTo_d[lo:lo + 1024, :].rearrange("(t p) s -> p t s", p=128),
                            xT[:, lo:lo + 1024].rearrange("p (t s) -> p t s", s=BS),
                        )

                # dual update, entirely in transposed space:
                #   v = Ax + u - y;  f = min(1, eps/(||v||+tiny))
                #   u' = v*(1-f);  zmu' = y - u' + v*f = y + v*(2f-1)
                vT = small.tile([128, BS], f32, tag="vT", bufs=2)
                nc.vector.tensor_add(vT[:], ax_ps[:], uyT[:])
                sq = small.tile([128, BS], f32, tag="sq", bufs=2)
                nc.vector.tensor_mul(sq[:], vT[:], vT[:])
                nrm2 = ps.tile([1, BS], f32, tag="tp", bufs=2)
                nc.tensor.matmul(nrm2[:], ones_col, sq[:], start=True, stop=True)
                nrm = small.tile([1, BS], f32, tag="nrm", bufs=2)
                nc.scalar.sqrt(nrm[:], nrm2[:])
                rec = small.tile([1, BS], f32, tag="rec", bufs=2)
                nc.vector.tensor_scalar_add(nrm[:], nrm[:], EPS_DIV)
                nc.vector.reciprocal(rec[:], nrm[:])
                hg = small.tile([1, 2 * BS], f32, tag="hg", bufs=2)
                # f = min(eps*rec, 1);  h = 2f-1 ; g = 1-f
                fs = small.tile([1, BS], f32, tag="fs", bufs=2)
                nc.vector.tensor_scalar(
                    fs[:], rec[:], float(eps), 1.0, op0=Alu.mult, op1=Alu.min,
                )
                nc.vector.tensor_scalar(
                    hg[:, 0:BS], fs[:], 2.0, 1.0, op0=Alu.mult, op1=Alu.subtract,
                )
                nc.vector.tensor_scalar(
                    hg[:, BS:2 * BS], fs[:], -1.0, 1.0, op0=Alu.mult, op1=Alu.add,
                )
                bc = ps.tile([128, 2 * BS], f32, tag="tp", bufs=2)
                nc.tensor.matmul(bc[:], ones_row, hg[:], start=True, stop=True)
                # u' = v*(1-f)
                uT_new = small.tile([128, BS], f32, tag="uT", bufs=2)
                nc.vector.tensor_mul(uT_new[:], vT[:], bc[:, BS:2 * BS])
                if last:
                    nc.sync.dma_start(uo_d[:], uT_new[:])
                else:
                    uyT = small.tile([128, BS], f32, tag="uyT", bufs=2)
                    nc.vector.tensor_sub(uyT[:], uT_new[:], yT[:])
                    # zmu' = y + v*(2f-1)
                    zmT = small.tile([128, BS], f32, tag="zmT", bufs=2)
                    nc.vector.tensor_mul(zmT[:], vT[:], bc[:, 0:BS])
                    zmuT = small.tile([128, BS], f32, tag="zmuT", bufs=2)
                    nc.vector.tensor_add(zmuT[:], zmT[:], yT[:])
                    sol = ps.tile([128, BS], f32, tag="sol", bufs=1)
                    nc.tensor.matmul(sol[:], HT_sb, zmuT[:], start=True, stop=False)

    nc.compile()
    return nc


def _host_prep(A, log_rho, log_epsilon):
    rho = float(np.exp(np.float64(np.asarray(log_rho))))
    eps = float(np.exp(np.float64(np.asarray(log_epsilon))))
    Ar = np.asarray(A[0], np.float64)
    Ai = np.asarray(A[1], np.float64)
    Gr = Ar @ Ar.T + Ai @ Ai.T
    Gi = Ai @ Ar.T - Ar @ Ai.T
    Sr = np.eye(M) / (rho + EPS_DIV) + Gr
    Sb = np.block([[Sr, -Gi], [Gi, Sr]])
    Sinv = np.linalg.inv(Sb)
    Gb = np.block([[Gr, -Gi], [Gi, Gr]])
    H = rho * (Sinv @ Gb)
    HT = np.ascontiguousarray(H.T, dtype=np.float32)
    SinvT = np.ascontiguousarray(Sinv.T, dtype=np.float32)
    return rho, eps, HT, SinvT


def _chunked_T(rT):
    """[N, BS] -> [128, N] with col t*BS+s = rT[t*128+p, s]."""
    return np.ascontiguousarray(
        rT.reshape(T, 128, BS).transpose(1, 0, 2).reshape(128, T * BS)
    )


def make_in_maps(r_n, y, u_in, A, log_rho, log_epsilon):
    rho, eps, HT, SinvT = _host_prep(A, log_rho, log_epsilon)
    A_f = np.asarray(A, np.float32)
    A_stack = np.ascontiguousarray(A_f.reshape(128, N))
    AT1_ch = _chunked_T(np.concatenate([A_f[0].T, A_f[1].T], axis=1))
    I128 = np.eye(128, dtype=np.float32)
    sm = np.ascontiguousarray(np.concatenate(
        [HT, SinvT, I128, np.ones((128, 128), np.float32)], axis=1
    ), np.float32)
    r_n = np.asarray(r_n, np.float32)
    y = np.asarray(y, np.float32)
    u_in = np.asarray(u_in, np.float32)

    in_maps = []
    for c in range(NCORES):
        sl = slice(c * BS, (c + 1) * BS)
        rTr_ch = _chunked_T(np.ascontiguousarray(r_n[sl, 0, :].T))
        rTi_ch = _chunked_T(np.ascontiguousarray(r_n[sl, 1, :].T))
        im = {
            "sm": sm,
            "yT": np.ascontiguousarray(y[sl].reshape(BS, 128).T),
            "u0T": np.ascontiguousarray(u_in[sl].reshape(BS, 128).T),
        }
        for k in range(4):
            im[f"As{k}"] = np.ascontiguousarray(
                A_stack[:, k * (N // 4):(k + 1) * (N // 4)])
        W = TG * 128
        for g in range(G):
            gs = slice(g * W, (g + 1) * W)
            im[f"g{g}"] = np.ascontiguousarray(np.concatenate(
                [AT1_ch[:, gs], rTr_ch[:, gs], rTi_ch[:, gs]], axis=1
            ))
        in_maps.append(im)
    return rho, eps, in_maps


def assemble_outputs(out_maps):
    xs = []
    us = []
    for c in range(NCORES):
        xs.append(np.asarray(out_maps[c]["xTo"]).T)  # [BS, N]
        us.append(np.asarray(out_maps[c]["uoT"]).T.reshape(BS, 2, M))
    x_r = np.concatenate(xs, axis=0)  # [1024, N]
    x = np.stack([x_r, np.zeros_like(x_r)], axis=1)
    u = np.concatenate(us, axis=0)
    return x.astype(np.float32), u.astype(np.float32)


def kernel(r_n, y, u_in, A, log_rho, log_epsilon, _trace=False):
    from concourse.bass_utils import run_bass_kernel_spmd

    rho, eps, in_maps = make_in_maps(r_n, y, u_in, A, log_rho, log_epsilon)
    key = (round(rho, 12), round(eps, 12))
    if key not in _BUILD_CACHE:
        _BUILD_CACHE[key] = build_bass(rho, eps)
    nc = _BUILD_CACHE[key]
    res = run_bass_kernel_spmd(
        nc, in_maps, core_ids=list(range(NCORES)), trace=_trace,
    )
    x, u = assemble_outputs(res.results)
    if _trace:
        kernel._last_exec_time_ns = res.exec_time_ns
        kernel._last_results = res
    return x, u
